# revision 1
# baseline (speedup 1.0000x reference)
"""CTRNN (6 unfolds) Trainium2 Bass kernel, data-parallel over 8 NeuronCores.

Math (per reference):
    w_x = fc_w[:, :512]; w_h = fc_w[:, 512:]
    xw  = x @ w_x^T + 0 (bias folded into tanh)
    repeat 6x:  f = tanh(xw + h @ w_h^T + b);  h = 0.9*h + 0.1*f

Device layout: everything transposed ([feature, batch]) so the recurrent
matmul needs no on-chip transposes.  Per core: batch shard of 2048.
h is kept in a rescaled representation H_t = h_t / 0.9^t so both the
xw-add and the state update are single fused scalar_tensor_tensor ops.
Matmuls run as float32r (full fp32 operands, reduced-precision multiply,
fp32 PSUM accumulate) which streams at 1 cycle/row for N=512.
"""

import numpy as np
from contextlib import ExitStack

import concourse.bass as bass
import concourse.tile as tile
import concourse.mybir as mybir
from concourse.bass_utils import run_bass_kernel_spmd


def _patch_tile_drain():
    """The walrus build in this image encodes at most one sync-wait on a
    Drain CTRL instruction; Tile's kernel-tail drain attaches one wait per
    outstanding proc and fails codegen ("Too many sync wait commands").
    Spread those waits across single-wait SP nops, then emit a bare drain."""
    if getattr(tile.TileContext, "_drain_split_patched", False):
        return
    from concourse.vector_clock import ScopedClock

    def _drain_and_barrier(self, tick_clock, wait_clock):
        nc = self.nc
        collector = nc.sync.nop(nofuse=True)
        wait_clock.add_sem_waits(
            collector.ins, ScopedClock({None: tick_clock.global_clock})
        )
        waits = list(collector.ins.sync_info.on_wait)
        del collector.ins.sync_info.on_wait[1:]
        for w in waits[1:]:
            nop = nc.sync.nop(nofuse=True)
            if nop.ins.sync_info is None:
                nop.ins.sync_info = mybir.SyncInfo(on_wait=[], on_update=[])
            nop.ins.sync_info.on_wait.append(w)
        nc.sync.drain()
        nc.all_engine_barrier()
        assert self.sems is not None
        popped = nc._tile_sem_poison_stack.pop()
        assert popped is self._sem_poison
        nc.clear_and_free_semaphores(list(self.sems.allocated().values()))
        nc.all_engine_barrier()

    tile.TileContext._drain_and_barrier = _drain_and_barrier
    tile.TileContext._drain_split_patched = True


_patch_tile_drain()


def _split_excess_waits_json(bir_json):
    """This image's walrus encodes at most ONE sync-wait per instruction
    (setupSyncWait: "Too many sync wait commands").  Tile attaches as many
    waits as deps require.  Hoist all but one wait of each instruction onto
    injected NoOps, placed just before it on the same engine."""
    import json as _json

    js = _json.loads(bir_json)
    n_split = 0
    for fn in js["functions"]:
        for blk in fn["blocks"]:
            out_insts = []
            for inst in blk["instructions"]:
                si = inst.get("sync_info") or {}
                ow = si.get("on_wait") or []
                if len(ow) > 1:
                    for w in ow[:-1]:
                        n_split += 1
                        nop = {
                            "name": f"I-ws{n_split}",
                            "opcode": "NoOp",
                            "engine": inst["engine"],
                            "ins": [],
                            "outs": [],
                            "sync_info": {"on_update": [], "on_wait": [w]},
                        }
                        if "debug" in inst:
                            nop["debug"] = inst["debug"]
                        out_insts.append(nop)
                    si["on_wait"] = [ow[-1]]
                out_insts.append(inst)
            blk["instructions"] = out_insts
    return _json.dumps(js).encode()


def _patch_compile_for_wait_cap():
    import concourse.bass_utils as _bu

    if getattr(_bu, "_wait_split_patched", False):
        return
    _orig = _bu._compile_bir_impl

    def _impl(bir_json, *args, **kwargs):
        return _orig(_split_excess_waits_json(bir_json), *args, **kwargs)

    _bu._compile_bir_impl = _impl
    _bu._wait_split_patched = True


_patch_compile_for_wait_cap()

B, D_IN, D_H = 16384, 512, 1024
N_CORES = 8
BS = B // N_CORES            # 2048 batch rows per core
UNFOLDS = 6
DT = 0.1
DECAY = 0.9                  # 1 - DT/TAU
CH = 512                     # batch chunk (matmul moving free dim)
NCH = BS // CH               # 4 chunks per core
KB = D_H // 128              # 8 hidden-dim k-blocks
KX = D_IN // 128             # 4 input-dim k-blocks
F32 = mybir.dt.float32
F32R = mybir.dt.float32r


def build_nc() -> bass.Bass:
    nc = bass.Bass()
    xT = nc.dram_tensor("xT", [D_IN, BS], F32, kind="ExternalInput")
    hT = nc.dram_tensor("hT", [D_H, BS], F32, kind="ExternalInput")
    wxT = nc.dram_tensor("wxT", [D_IN, D_H], F32, kind="ExternalInput")
    whT = nc.dram_tensor("whT", [D_H, D_H], F32, kind="ExternalInput")
    bias = nc.dram_tensor("bias", [128, KB], F32, kind="ExternalInput")
    out = nc.dram_tensor("out", [D_H, BS], F32, kind="ExternalOutput")

    with tile.TileContext(nc) as tc, ExitStack() as ctx:
        persist = ctx.enter_context(tc.tile_pool(name="persist", bufs=1))
        psum_pool = ctx.enter_context(tc.tile_pool(name="psum", bufs=8, space="PSUM"))

        # --- persistent SBUF state ---
        # h, one tile per batch chunk: [128, KB*CH]; k-block jb at cols jb*CH
        h_sb = [
            persist.tile([128, KB * CH], F32R, name=f"h_sb{c}", tag=f"h_sb{c}")
            for c in range(NCH)
        ]
        # w_h^T: [128, KB*D_H]; k-block jb at cols jb*D_H
        wh_sb = persist.tile([128, KB * D_H], F32R, name="wh_sb", tag="wh_sb")
        b_sb = persist.tile([128, KB], F32, name="b_sb", tag="b_sb")
        # xw resident in SBUF as bf16, same [feature-block, chunk] layout as h
        BF16 = mybir.dt.bfloat16
        xw_sb = [
            persist.tile([128, KB * CH], BF16, name=f"xw_sb{c}", tag=f"xw_sb{c}")
            for c in range(NCH)
        ]

        nc.sync.dma_start(out=b_sb[:], in_=bias[:, :])

        # --- phase 1: xw = x @ w_x^T, staged to DRAM (fp32 exact) ---
        # Load order matters for the head: wx + x chunk 0 gate the first
        # matmul; h/wh aren't needed until the unfold phase and load later.
        with tc.tile_pool(name="xpre", bufs=1) as xpool, \
             tc.tile_pool(name="wxpre", bufs=1) as wxpool:
            wx_sb = wxpool.tile([128, KX * D_H], F32R, name="wx_sb", tag="wx_sb")
            x_sbs = [
                xpool.tile([128, KX * CH], F32R, name="x_sb", tag=f"x_sb{c}")
                for c in range(NCH)
            ]
            # per-k-block loads, wx/x0 interleaved: with subtile deps the
            # first matmul starts once block 0 of each is resident.
            for kb in range(KX):
                nc.gpsimd.dma_start(
                    out=wx_sb[:, kb * D_H:(kb + 1) * D_H],
                    in_=wxT[kb * 128:(kb + 1) * 128, :],
                )
                nc.gpsimd.dma_start(
                    out=x_sbs[0][:, kb * CH:(kb + 1) * CH],
                    in_=xT[kb * 128:(kb + 1) * 128, 0:CH],
                )
            for c in range(1, NCH):
                nc.gpsimd.dma_start(
                    out=x_sbs[c][:].rearrange("p (kb c) -> p kb c", c=CH),
                    in_=xT[:, c * CH:(c + 1) * CH].rearrange("(kb p) c -> p kb c", p=128),
                )
            # recurrent-phase loads queue behind every precompute gate; they
            # have the whole precompute to land.
            nc.gpsimd.dma_start(
                out=wh_sb[:].rearrange("p (jb h) -> p jb h", h=D_H),
                in_=whT[:, :].rearrange("(jb p) h -> p jb h", p=128),
            )
            for hc in range(NCH):
                nc.gpsimd.dma_start(
                    out=h_sb[hc][:].rearrange("p (jb c) -> p jb c", c=CH),
                    in_=hT[:, hc * CH:(hc + 1) * CH].rearrange("(jb p) c -> p jb c", p=128),
                )
            for c in range(NCH):
                x_sb = x_sbs[c]
                for p in range(KB):
                    ps = psum_pool.tile([128, CH], F32, name="ps", tag="ps")
                    for kb in range(KX):
                        nc.tensor.matmul(
                            ps[:],
                            wx_sb[:, kb * D_H + p * 128: kb * D_H + (p + 1) * 128],
                            x_sb[:, kb * CH:(kb + 1) * CH],
                            start=(kb == 0),
                            stop=(kb == KX - 1),
                        )
                    nc.vector.tensor_copy(
                        xw_sb[c][:, p * CH:(p + 1) * CH], ps[:]
                    )

        # --- phase 2: unfold loop ---
        # opened after xpre/wxpre release so the allocator reuses their space
        fpool = ctx.enter_context(tc.tile_pool(name="fpool", bufs=3))
        stage_pool = ctx.enter_context(tc.tile_pool(name="stage", bufs=6))
        sigma = 1.0  # SBUF holds H_t = h_t / sigma
        for t in range(UNFOLDS):
            last = t == UNFOLDS - 1
            upd = DT / (sigma * DECAY)  # coefficient on f for the H update
            for c in range(NCH):
                if not last:
                    # f for the whole chunk, written per H-tile; the h update
                    # must only run after every matmul group has read old h
                    # (Jacobi, not Gauss-Seidel).
                    f_ch = fpool.tile([128, KB * CH], F32, name="f_ch", tag="f_ch", bufs=2)
                for p in range(KB):
                    if last:
                        # h6 = 0.9*sigma*H + 0.1*f, per tile, stored as soon
                        # as ready; the 0.9*sigma*H part has no dependence on
                        # this step's matmuls and runs early.
                        hs = stage_pool.tile([128, CH], F32, name="hs", tag="st")
                        nc.vector.tensor_scalar_mul(
                            hs[:], h_sb[c][:, p * CH:(p + 1) * CH],
                            float(DECAY * sigma),
                        )
                    ps = psum_pool.tile([128, CH], F32, name="ps", tag="ps")
                    for jb in range(KB):
                        nc.tensor.matmul(
                            ps[:],
                            wh_sb[:, jb * D_H + p * 128: jb * D_H + (p + 1) * 128],
                            h_sb[c][:, jb * CH:(jb + 1) * CH],
                            start=(jb == 0),
                            stop=(jb == KB - 1),
                        )
                    # z = sigma * (W @ H) + xw  (in place on psum)
                    nc.vector.scalar_tensor_tensor(
                        ps[:], ps[:], float(sigma),
                        xw_sb[c][:, p * CH:(p + 1) * CH],
                        op0=mybir.AluOpType.mult, op1=mybir.AluOpType.add,
                    )
                    if last:
                        f_t = fpool.tile([128, CH], F32, name="f_t", tag="f_t")
                        nc.scalar.activation(
                            f_t[:], ps[:], mybir.ActivationFunctionType.Tanh,
                            bias=b_sb[:, p:p + 1], scale=1.0,
                        )
                        nc.vector.scalar_tensor_tensor(
                            hs[:], f_t[:], float(DT), hs[:],
                            op0=mybir.AluOpType.mult, op1=mybir.AluOpType.add,
                        )
                        nc.sync.dma_start(
                            out=out[p * 128:(p + 1) * 128, c * CH:(c + 1) * CH],
                            in_=hs[:],
                        )
                    else:
                        nc.scalar.activation(
                            f_ch[:, p * CH:(p + 1) * CH], ps[:],
                            mybir.ActivationFunctionType.Tanh,
                            bias=b_sb[:, p:p + 1], scale=1.0,
                        )
                if not last:
                    # H += upd * f, whole chunk in one op (in place on h)
                    nc.vector.scalar_tensor_tensor(
                        h_sb[c][:], f_ch[:], float(upd), h_sb[c][:],
                        op0=mybir.AluOpType.mult, op1=mybir.AluOpType.add,
                    )
            sigma *= DECAY
    return nc


_NC_CACHE = {}


def _get_nc() -> bass.Bass:
    if "nc" not in _NC_CACHE:
        _NC_CACHE["nc"] = build_nc()
    return _NC_CACHE["nc"]


def make_in_maps(x, h, fc_w, fc_b):
    x = np.asarray(x, dtype=np.float32)
    h = np.asarray(h, dtype=np.float32)
    fc_w = np.asarray(fc_w, dtype=np.float32)
    fc_b = np.asarray(fc_b, dtype=np.float32)
    xT = np.ascontiguousarray(x.T)                    # [D_IN, B]
    hT = np.ascontiguousarray(h.T)                    # [D_H, B]
    wxT = np.ascontiguousarray(fc_w[:, :D_IN].T)      # [D_IN, D_H]
    whT = np.ascontiguousarray(fc_w[:, D_IN:].T)      # [D_H, D_H]
    bias = np.ascontiguousarray(fc_b.reshape(KB, 128).T)  # [128, KB]
    in_maps = []
    for i in range(N_CORES):
        sl = slice(i * BS, (i + 1) * BS)
        in_maps.append({
            "xT": np.ascontiguousarray(xT[:, sl]),
            "hT": np.ascontiguousarray(hT[:, sl]),
            "wxT": wxT,
            "whT": whT,
            "bias": bias,
        })
    return in_maps


def gather_out(results):
    outT = np.concatenate([results[i]["out"] for i in range(N_CORES)], axis=1)
    return np.ascontiguousarray(outT.T)  # [B, D_H]


def kernel(x, h, fc_w, fc_b):
    nc = _get_nc()
    in_maps = make_in_maps(x, h, fc_w, fc_b)
    res = run_bass_kernel_spmd(nc, in_maps, list(range(N_CORES)))
    out = gather_out(res.results)
    return (out, out)


if __name__ == "__main__":
    rng = np.random.default_rng(0)
    x = rng.standard_normal((B, D_IN), dtype=np.float32)
    h = rng.standard_normal((B, D_H), dtype=np.float32)
    fc_w = rng.standard_normal((D_H, D_IN + D_H), dtype=np.float32) / np.sqrt(D_IN + D_H)
    fc_b = np.zeros((D_H,), dtype=np.float32)
    o, _ = kernel(x, h, fc_w, fc_b)
    print(o.shape, o.dtype)



# revision 4
# speedup vs baseline: 1.1803x; 1.1803x over previous
"""CTRNN (6 unfolds) Trainium2 Bass kernel, data-parallel over 8 NeuronCores.

Math (per reference):
    w_x = fc_w[:, :512]; w_h = fc_w[:, 512:]
    xw  = x @ w_x^T + b
    repeat 6x:  f_t = tanh(xw + h_t @ w_h^T);  h_{t+1} = 0.9*h_t + 0.1*f_t

Reformulated in pre-activation space so the recurrent matmul can run in
fp8 (DoubleRow, 2x PE throughput) with its quantization error damped 10x:
    y_t := w_h @ h_t        (state, [feature, batch], scaled Y = 64*y, bf16)
    f_t  = tanh(y_t + xw + b)
    y_{t+1} = 0.9*y_t + 0.1*(w_h @ f_t)     <- fp8 matmul, 0.1-damped
    h_6  = 0.9^6*h_0 + sum_t 0.1*0.9^(5-t)*f_t  (accumulator A on device;
                                                 the 0.9^6*h_0 axpy on host)
Weights for the recurrent matmuls are host-prequantized e4m3(6.4*w_h^T) so
the PSUM drain is a single in-place scalar_tensor_tensor Y = 0.9*Y + P.
y_0 is computed with a bf16 matmul (full-scale error path), xw in bf16.

Device layout: everything transposed ([feature, batch]). Per core: batch
shard of 2048, processed as 4 chunks of 512 (PSUM-bank-sized moving dim).
"""

import numpy as np
from contextlib import ExitStack

import ml_dtypes

import concourse.bass as bass
import concourse.tile as tile
import concourse.mybir as mybir
from concourse.bass_utils import run_bass_kernel_spmd


def _patch_tile_drain():
    """The walrus build in this image encodes at most one sync-wait on a
    Drain CTRL instruction; Tile's kernel-tail drain attaches one wait per
    outstanding proc and fails codegen ("Too many sync wait commands").
    Spread those waits across single-wait SP nops, then emit a bare drain."""
    if getattr(tile.TileContext, "_drain_split_patched", False):
        return
    from concourse.vector_clock import ScopedClock

    def _drain_and_barrier(self, tick_clock, wait_clock):
        nc = self.nc
        collector = nc.sync.nop(nofuse=True)
        wait_clock.add_sem_waits(
            collector.ins, ScopedClock({None: tick_clock.global_clock})
        )
        waits = list(collector.ins.sync_info.on_wait)
        del collector.ins.sync_info.on_wait[1:]
        for w in waits[1:]:
            nop = nc.sync.nop(nofuse=True)
            if nop.ins.sync_info is None:
                nop.ins.sync_info = mybir.SyncInfo(on_wait=[], on_update=[])
            nop.ins.sync_info.on_wait.append(w)
        nc.sync.drain()
        nc.all_engine_barrier()
        assert self.sems is not None
        popped = nc._tile_sem_poison_stack.pop()
        assert popped is self._sem_poison
        nc.clear_and_free_semaphores(list(self.sems.allocated().values()))
        nc.all_engine_barrier()

    tile.TileContext._drain_and_barrier = _drain_and_barrier
    tile.TileContext._drain_split_patched = True


_patch_tile_drain()


def _split_excess_waits_json(bir_json):
    """This image's walrus encodes at most ONE sync-wait per instruction
    (setupSyncWait: "Too many sync wait commands").  Tile attaches as many
    waits as deps require.  Hoist all but one wait of each instruction onto
    injected NoOps, placed just before it on the same engine."""
    import json as _json

    js = _json.loads(bir_json)
    n_split = 0
    for fn in js["functions"]:
        for blk in fn["blocks"]:
            out_insts = []
            for inst in blk["instructions"]:
                si = inst.get("sync_info") or {}
                ow = si.get("on_wait") or []
                if len(ow) > 1:
                    for w in ow[:-1]:
                        n_split += 1
                        nop = {
                            "name": f"I-ws{n_split}",
                            "opcode": "NoOp",
                            "engine": inst["engine"],
                            "ins": [],
                            "outs": [],
                            "sync_info": {"on_update": [], "on_wait": [w]},
                        }
                        if "debug" in inst:
                            nop["debug"] = inst["debug"]
                        out_insts.append(nop)
                    si["on_wait"] = [ow[-1]]
                out_insts.append(inst)
            blk["instructions"] = out_insts
    return _json.dumps(js).encode()


def _patch_compile_for_wait_cap():
    import concourse.bass_utils as _bu

    if getattr(_bu, "_wait_split_patched", False):
        return
    _orig = _bu._compile_bir_impl

    def _impl(bir_json, *args, **kwargs):
        return _orig(_split_excess_waits_json(bir_json), *args, **kwargs)

    _bu._compile_bir_impl = _impl
    _bu._wait_split_patched = True


_patch_compile_for_wait_cap()

B, D_IN, D_H = 16384, 512, 1024
N_CORES = 8
BS = B // N_CORES            # 2048 batch rows per core
UNFOLDS = 6
DT = 0.1
DECAY = 0.9                  # 1 - DT/TAU
CH = 512                     # batch chunk (matmul moving free dim)
NCH = BS // CH               # 4 chunks per core
KB = D_H // 128              # 8 hidden-dim k-blocks
KX = D_IN // 128             # 4 input-dim k-blocks
NU = KB // 2                 # 4 DoubleRow k-block pairs
KSCALE = 64.0                # Y = 64*y  (weights carry 0.1*64 = 6.4)
F32 = mybir.dt.float32
BF16 = mybir.dt.bfloat16
F8 = mybir.dt.float8e4
MUL = mybir.AluOpType.mult
ADD = mybir.AluOpType.add
DR = mybir.MatmulPerfMode.DoubleRow


def build_nc() -> bass.Bass:
    nc = bass.Bass()
    x16 = nc.dram_tensor("x16", [D_IN, BS], BF16, kind="ExternalInput")
    h16 = nc.dram_tensor("h16", [D_H, BS], BF16, kind="ExternalInput")
    wx16 = nc.dram_tensor("wx16", [D_IN, D_H], BF16, kind="ExternalInput")
    wh16 = nc.dram_tensor("wh16", [D_H, D_H], BF16, kind="ExternalInput")
    # paired DoubleRow layout: [kpart, (u, p, two, col)] = [128, 8192]
    wh8p = nc.dram_tensor("wh8p", [128, KB * D_H], F8, kind="ExternalInput")
    biasd = nc.dram_tensor("bias", [128, KB], F32, kind="ExternalInput")
    aout = nc.dram_tensor("aout", [D_H, BS], BF16, kind="ExternalOutput")

    with tile.TileContext(nc) as tc, ExitStack() as ctx:
        persist = ctx.enter_context(tc.tile_pool(name="persist", bufs=1))
        psum_pool = ctx.enter_context(tc.tile_pool(name="psum", bufs=8, space="PSUM"))

        bias_sb = persist.tile([128, KB], F32, name="b_sb", tag="b_sb")
        wh8_sb = persist.tile([128, KB * D_H], F8, name="wh8", tag="wh8")
        xwb = [persist.tile([128, KB * CH], BF16, name=f"xwb{c}", tag=f"xwb{c}")
               for c in range(NCH)]
        Y = [persist.tile([128, KB * CH], BF16, name=f"Y{c}", tag=f"Y{c}")
             for c in range(NCH)]
        A = [persist.tile([128, KB * CH], BF16, name=f"A{c}", tag=f"A{c}")
             for c in range(NCH)]
        f8 = [persist.tile([128, KB * CH], F8, name=f"f8_{c}", tag=f"f8_{c}")
              for c in range(NCH)]

        with tc.tile_pool(name="pre", bufs=1) as pre:
            wx_sb = pre.tile([128, KX * D_H], BF16, name="wx_sb", tag="wx_sb")
            x_sb = [pre.tile([128, KX * CH], BF16, name="x_sb", tag=f"x_sb{c}")
                    for c in range(NCH)]
            wh16_sb = pre.tile([128, KB * D_H], BF16, name="wh16", tag="wh16")
            h16_sb = [pre.tile([128, KB * CH], BF16, name="h16", tag=f"h16_{c}")
                      for c in range(NCH)]

            nc.sync.dma_start(out=bias_sb[:], in_=biasd[:, :])
            # head-critical loads first: wx + x chunk 0 gate the first matmul
            for kb in range(KX):
                nc.gpsimd.dma_start(
                    out=wx_sb[:, kb * D_H:(kb + 1) * D_H],
                    in_=wx16[kb * 128:(kb + 1) * 128, :],
                )
                nc.gpsimd.dma_start(
                    out=x_sb[0][:, kb * CH:(kb + 1) * CH],
                    in_=x16[kb * 128:(kb + 1) * 128, 0:CH],
                )
            for c in range(1, NCH):
                nc.gpsimd.dma_start(
                    out=x_sb[c][:].rearrange("q (kb n) -> q kb n", n=CH),
                    in_=x16[:, c * CH:(c + 1) * CH].rearrange(
                        "(kb q) n -> q kb n", q=128),
                )
            # y0-phase + step-phase loads; they have all of phase 1 to land
            nc.gpsimd.dma_start(
                out=wh16_sb[:].rearrange("q (jb m) -> q jb m", m=D_H),
                in_=wh16[:, :].rearrange("(jb q) m -> q jb m", q=128),
            )
            for c in range(NCH):
                nc.gpsimd.dma_start(
                    out=h16_sb[c][:].rearrange("q (jb n) -> q jb n", n=CH),
                    in_=h16[:, c * CH:(c + 1) * CH].rearrange(
                        "(jb q) n -> q jb n", q=128),
                )
            nc.gpsimd.dma_start(out=wh8_sb[:], in_=wh8p[:, :])

            # --- phase 1: xwb = bf16(x @ w_x^T + b), bf16 matmul ---
            for c in range(NCH):
                for p in range(KB):
                    ps = psum_pool.tile([128, CH], F32, name="ps", tag="ps")
                    for kb in range(KX):
                        nc.tensor.matmul(
                            ps[:],
                            wx_sb[:, kb * D_H + p * 128: kb * D_H + (p + 1) * 128],
                            x_sb[c][:, kb * CH:(kb + 1) * CH],
                            start=(kb == 0),
                            stop=(kb == KX - 1),
                        )
                    nc.scalar.add(
                        xwb[c][:, p * CH:(p + 1) * CH], ps[:],
                        add=bias_sb[:, p:p + 1],
                    )

            # --- phase 2: Y = 64 * (w_h @ h_0), bf16 matmul ---
            for c in range(NCH):
                for p in range(KB):
                    ps = psum_pool.tile([128, CH], F32, name="ps", tag="ps")
                    for jb in range(KB):
                        nc.tensor.matmul(
                            ps[:],
                            wh16_sb[:, jb * D_H + p * 128: jb * D_H + (p + 1) * 128],
                            h16_sb[c][:, jb * CH:(jb + 1) * CH],
                            start=(jb == 0),
                            stop=(jb == KB - 1),
                        )
                    nc.vector.tensor_scalar_mul(
                        Y[c][:, p * CH:(p + 1) * CH], ps[:], KSCALE
                    )

        # --- phase 3: unfold loop (fp8 DoubleRow recurrent matmuls) ---
        tmp_pool = ctx.enter_context(tc.tile_pool(name="tmp", bufs=2))
        for t in range(UNFOLDS):
            ct = float(DT * DECAY ** (UNFOLDS - 1 - t))
            for c in range(NCH):
                tmp = tmp_pool.tile([128, KB * CH], F32, name="tmp", tag="tmp")
                nc.vector.scalar_tensor_tensor(
                    tmp[:], Y[c][:], float(1.0 / KSCALE), xwb[c][:],
                    op0=MUL, op1=ADD,
                )
                nc.scalar.activation(
                    f8[c][:], tmp[:], mybir.ActivationFunctionType.Tanh,
                    bias=0.0, scale=1.0,
                )
                if t == 0:
                    nc.vector.tensor_scalar_mul(A[c][:], f8[c][:], ct)
                else:
                    nc.vector.scalar_tensor_tensor(
                        A[c][:], f8[c][:], ct, A[c][:], op0=MUL, op1=ADD,
                    )
                if t < UNFOLDS - 1:
                    for p in range(KB):
                        ps = psum_pool.tile([128, CH], F32, name="ps", tag="ps")
                        for u in range(NU):
                            off = (u * KB + p) * 256
                            nc.tensor.matmul(
                                ps[:],
                                wh8_sb[:, off:off + 256].rearrange(
                                    "q (two m) -> q two m", two=2),
                                f8[c][:, (2 * u) * CH:(2 * u + 2) * CH].rearrange(
                                    "q (two n) -> q two n", two=2),
                                start=(u == 0),
                                stop=(u == NU - 1),
                                perf_mode=DR,
                            )
                        nc.vector.scalar_tensor_tensor(
                            Y[c][:, p * CH:(p + 1) * CH],
                            Y[c][:, p * CH:(p + 1) * CH],
                            float(DECAY), ps[:], op0=MUL, op1=ADD,
                        )
                else:
                    nc.sync.dma_start(
                        out=aout[:, c * CH:(c + 1) * CH].rearrange(
                            "(jb q) n -> q jb n", q=128),
                        in_=A[c][:].rearrange("q (jb n) -> q jb n", n=CH),
                    )
    return nc


_NC_CACHE = {}


def _get_nc() -> bass.Bass:
    if "nc" not in _NC_CACHE:
        _NC_CACHE["nc"] = build_nc()
    return _NC_CACHE["nc"]


def make_in_maps(x, h, fc_w, fc_b):
    x = np.asarray(x, dtype=np.float32)
    h = np.asarray(h, dtype=np.float32)
    fc_w = np.asarray(fc_w, dtype=np.float32)
    fc_b = np.asarray(fc_b, dtype=np.float32)
    xT = np.ascontiguousarray(x.T).astype(ml_dtypes.bfloat16)      # [D_IN, B]
    hT = np.ascontiguousarray(h.T).astype(ml_dtypes.bfloat16)      # [D_H, B]
    wx16 = np.ascontiguousarray(fc_w[:, :D_IN].T).astype(ml_dtypes.bfloat16)
    whT = np.ascontiguousarray(fc_w[:, D_IN:].T)                   # [D_H, D_H]
    wh16 = whT.astype(ml_dtypes.bfloat16)
    # DoubleRow-paired fp8 weights: [k, h] -> [kpart, (u, p, two, col)]
    w8 = (DT * KSCALE * whT).astype(ml_dtypes.float8_e4m3)
    wh8p = np.ascontiguousarray(
        w8.reshape(NU, 2, 128, KB, 128).transpose(2, 0, 3, 1, 4)
        .reshape(128, KB * D_H))
    bias = np.ascontiguousarray(fc_b.reshape(KB, 128).T)           # [128, KB]
    in_maps = []
    for i in range(N_CORES):
        sl = slice(i * BS, (i + 1) * BS)
        in_maps.append({
            "x16": np.ascontiguousarray(xT[:, sl]),
            "h16": np.ascontiguousarray(hT[:, sl]),
            "wx16": wx16,
            "wh16": wh16,
            "wh8p": wh8p,
            "bias": bias,
        })
    return in_maps


def gather_out(results, h):
    # device returns A = sum_t 0.1*0.9^(5-t) f_t  ([D_H, BS] bf16 per core);
    # finish h_6 = 0.9^6 h_0 + A here (h_0 is already on the host in f32)
    aT = np.concatenate(
        [results[i]["aout"] for i in range(N_CORES)], axis=1
    ).astype(np.float32)                                           # [D_H, B]
    out = (DECAY ** UNFOLDS) * np.asarray(h, dtype=np.float32) + aT.T
    return np.ascontiguousarray(out)                               # [B, D_H]


def kernel(x, h, fc_w, fc_b):
    nc = _get_nc()
    in_maps = make_in_maps(x, h, fc_w, fc_b)
    res = run_bass_kernel_spmd(nc, in_maps, list(range(N_CORES)))
    out = gather_out(res.results, h)
    return (out, out)


if __name__ == "__main__":
    rng = np.random.default_rng(0)
    x = rng.standard_normal((B, D_IN), dtype=np.float32)
    h = rng.standard_normal((B, D_H), dtype=np.float32)
    fc_w = rng.standard_normal((D_H, D_IN + D_H), dtype=np.float32) / np.sqrt(D_IN + D_H)
    fc_b = np.zeros((D_H,), dtype=np.float32)
    o, _ = kernel(x, h, fc_w, fc_b)
    print(o.shape, o.dtype)
